# revision 6
# baseline (speedup 1.0000x reference)
"""Trainium2 Bass kernel for nn_Attention (general-mode attention energies + softmax).

Math: energies[b,l] = sum_h (enc[b,l,:].W[h,:] + bias[h]) * hx[b,h]
               = enc[b,l,:] . v[b,:] + (hx[b].bias)      with v = hx @ W
The per-batch constant hx[b].bias cancels in the softmax, so the bias input is
unused.  This turns the reference's [B*L,1024]x[1024,1024] matmul into a tiny
[B,1024]x[1024,1024] matmul plus a batched dot-product against the streamed
encoder outputs, making the kernel HBM-bandwidth-bound.

Sharding: data-parallel over batch B=32 across 8 cores (4 batches each); W is
replicated.  No collectives needed.

Per-core device graph (Tile framework):
  1. v = hxT.T @ W on TensorE (contraction over h, 128-chunks).
  2. v bounced via DRAM and broadcast across 128 partitions.
  3. Stream enc in [128, 4, 1024] megatiles; fused DVE tensor_tensor_reduce
     gives energies[l] = sum_e enc[l,e]*v[e] (one pass per [128,1024] tile).
  4. PE-transpose energies [128,64] -> [64,128] so each PSUM row holds a
     contiguous 128-long l-range; softmax with a single global max (softmax is
     shift-invariant, so one max across all 4 batches is exact), ScalarE exp
     with fused per-row accumulation, per-batch denominators via a 0/1
     group-indicator matmul, then one contiguous DMA out.
"""

import sys

import numpy as np

if "/opt/trn_rl_repo" not in sys.path:
    sys.path.insert(0, "/opt/trn_rl_repo")

B, L, H = 32, 2048, 1024
N_CORES = 8
B_LOC = B // N_CORES  # 4 batches per core
NT = L // 128  # 16 l-tiles of 128 per batch
TG = 4  # l-tiles per DMA megatile
NMEGA = NT // TG  # 4 megatiles per batch

_CACHE = {}


def _build_nc():
    import concourse.bacc as bacc
    import concourse.bass as bass
    import concourse.tile as tile
    from concourse import mybir
    from concourse.masks import make_identity

    f32 = mybir.dt.float32
    Alu = mybir.AluOpType

    nc = bacc.Bacc(target_bir_lowering=False, debug=False)
    enc = nc.declare_dram_parameter("enc", [B_LOC * L, H], f32, isOutput=False)
    hxT = nc.declare_dram_parameter("hxT", [H, B_LOC], f32, isOutput=False)
    w = nc.declare_dram_parameter("w", [H, H], f32, isOutput=False)
    gmat = nc.declare_dram_parameter("gmat", [B_LOC * NT, B_LOC], f32, isOutput=False)
    gmatT = nc.declare_dram_parameter("gmatT", [B_LOC, B_LOC * NT], f32, isOutput=False)
    out = nc.declare_dram_parameter("out", [B_LOC, L], f32, isOutput=True)
    v_dram = nc.dram_tensor("v_bounce", [B_LOC, H], f32)

    with (
        tile.TileContext(nc) as tc,
        tc.tile_pool(name="consts", bufs=1) as consts,
        tc.tile_pool(name="wpool", bufs=1) as wpool,
        tc.tile_pool(name="encp", bufs=3) as encp,
        tc.tile_pool(name="scratch", bufs=2) as scratch,
        tc.tile_pool(name="small", bufs=1) as small,
        tc.tile_pool(name="psA", bufs=1, space="PSUM") as psA,
        tc.tile_pool(name="psB", bufs=1, space="PSUM") as psB,
    ):
        # ---- constants ----
        ident = consts.tile([128, 128], f32)
        make_identity(nc, ident)

        ones64 = consts.tile([1, 64], f32)
        nc.vector.memset(ones64, 1.0)

        # G[j, b] = 1 if j // NT == b  (row j = (b, t) pair); GT = G.T
        # (host-provided: engine APs can't start at partition offsets like 16)
        G = consts.tile([B_LOC * NT, B_LOC], f32)
        GT = consts.tile([B_LOC, B_LOC * NT], f32)
        nc.sync.dma_start(out=G, in_=gmat[:, :])
        nc.sync.dma_start(out=GT, in_=gmatT[:, :])

        # ---- phase 1: v = hx @ W ----
        hxT_sb = consts.tile([128, 8, B_LOC], f32)
        nc.sync.dma_start(out=hxT_sb, in_=hxT.rearrange("(c p) b -> p c b", p=128))

        w_sb = wpool.tile([128, 8, H], f32)
        nc.sync.dma_start(out=w_sb, in_=w.rearrange("(c p) e -> p c e", p=128))

        v_ps = psA.tile([B_LOC, H], f32)
        for half in range(2):
            sl = slice(half * 512, (half + 1) * 512)
            for c in range(8):
                nc.tensor.matmul(
                    v_ps[:, sl],
                    lhsT=hxT_sb[:, c, :],
                    rhs=w_sb[:, c, sl],
                    start=(c == 0),
                    stop=(c == 7),
                )
        v_sb = small.tile([B_LOC, H], f32)
        nc.vector.tensor_copy(v_sb, v_ps)
        nc.sync.dma_start(out=v_dram[:, :], in_=v_sb)

        # broadcast each batch's v across all 128 partitions (DRAM src, step-0)
        vb = consts.tile([128, B_LOC, H], f32)
        for bi in range(B_LOC):
            src = v_dram[bi : bi + 1, :]
            bcast = bass.AP(
                tensor=src.tensor, offset=src.offset, ap=[[0, 128]] + list(src.ap[1:])
            )
            nc.gpsimd.dma_start(out=vb[:, bi, :], in_=bcast)

        # ---- phase 2: energies via fused multiply+reduce on DVE ----
        energies = small.tile([128, B_LOC * NT], f32)  # col = b*NT + t
        for bi in range(B_LOC):
            for g in range(NMEGA):
                r0 = bi * L + g * (TG * 128)
                mt = encp.tile([128, TG, H], f32)
                nc.sync.dma_start(
                    out=mt,
                    in_=enc[r0 : r0 + TG * 128, :].rearrange("(j p) e -> p j e", p=128),
                )
                for j in range(TG):
                    t = g * TG + j
                    sc = scratch.tile([128, H], f32)
                    # out = (enc * 1.0) * v ; accum_out = per-partition sum
                    # (standard InstTensorScalarPtr — the custom DVE
                    # tensor_tensor_reduce is not runnable on this runtime)
                    nc.vector.scalar_tensor_tensor(
                        out=sc,
                        in0=mt[:, j, :],
                        scalar=1.0,
                        in1=vb[:, bi, :],
                        op0=Alu.mult,
                        op1=Alu.mult,
                        accum_out=energies[:, bi * NT + t : bi * NT + t + 1],
                    )

        # ---- phase 3: transpose + softmax ----
        eT_ps = psB.tile([B_LOC * NT, 128], f32)  # row j=(b,t) holds l in [t*128,(t+1)*128)
        nc.tensor.transpose(eT_ps, energies, ident)

        rowmax = small.tile([B_LOC * NT, 1], f32)
        nc.vector.reduce_max(out=rowmax, in_=eT_ps, axis=mybir.AxisListType.X)
        rmT_ps = psA.tile([1, B_LOC * NT], f32)
        nc.tensor.transpose(rmT_ps, rowmax, ident[: B_LOC * NT, : B_LOC * NT])
        gmax = small.tile([1, 1], f32)
        nc.vector.reduce_max(out=gmax, in_=rmT_ps, axis=mybir.AxisListType.X)
        ngmax = small.tile([1, 1], f32)
        nc.vector.tensor_scalar_mul(ngmax, gmax, -1.0)
        nm_ps = psA.tile([B_LOC * NT, 1], f32)
        nc.tensor.matmul(nm_ps, lhsT=ones64, rhs=ngmax, start=True, stop=True)
        nm_sb = small.tile([B_LOC * NT, 1], f32)
        nc.vector.tensor_copy(nm_sb, nm_ps)

        exps = small.tile([B_LOC * NT, 128], f32)
        rowsum = small.tile([B_LOC * NT, 1], f32)
        nc.scalar.activation(
            out=exps,
            in_=eT_ps,
            func=mybir.ActivationFunctionType.Exp,
            bias=nm_sb,
            scale=1.0,
            accum_out=rowsum,
        )

        den_ps = psA.tile([B_LOC, 1], f32)
        nc.tensor.matmul(den_ps, lhsT=G, rhs=rowsum, start=True, stop=True)
        rden = small.tile([B_LOC, 1], f32)
        nc.vector.reciprocal(rden, den_ps)
        rr_ps = psA.tile([B_LOC * NT, 1], f32)
        nc.tensor.matmul(rr_ps, lhsT=GT, rhs=rden, start=True, stop=True)
        rr_sb = small.tile([B_LOC * NT, 1], f32)
        nc.vector.tensor_copy(rr_sb, rr_ps)

        final = small.tile([B_LOC * NT, 128], f32)
        nc.vector.tensor_scalar_mul(final, exps, rr_sb)
        nc.sync.dma_start(out=out.rearrange("b (t p) -> (b t) p", p=128), in_=final)

    return nc


def get_nc():
    if "nc" not in _CACHE:
        nc = _build_nc()
        if not nc.is_finalized():
            nc.finalize()
        _CACHE["nc"] = nc
    return _CACHE["nc"]


def make_in_maps(hx, encoder_outputs, W):
    in_maps = []
    w = np.ascontiguousarray(W, dtype=np.float32)
    gmat = np.zeros((B_LOC * NT, B_LOC), dtype=np.float32)
    for bi in range(B_LOC):
        gmat[bi * NT : (bi + 1) * NT, bi] = 1.0
    gmatT = np.ascontiguousarray(gmat.T)
    for c in range(N_CORES):
        rows = slice(c * B_LOC, (c + 1) * B_LOC)
        in_maps.append(
            {
                "enc": np.ascontiguousarray(
                    encoder_outputs[rows], dtype=np.float32
                ).reshape(B_LOC * L, H),
                "hxT": np.ascontiguousarray(hx[rows].T, dtype=np.float32),
                "w": w,
                "gmat": gmat,
                "gmatT": gmatT,
            }
        )
    return in_maps


def kernel(hx, encoder_outputs, W, b, **_unused):
    from concourse.bass_utils import run_bass_kernel_spmd

    nc = get_nc()
    in_maps = make_in_maps(
        np.asarray(hx, dtype=np.float32),
        np.asarray(encoder_outputs, dtype=np.float32),
        np.asarray(W, dtype=np.float32),
    )
    res = run_bass_kernel_spmd(nc, in_maps, core_ids=list(range(N_CORES)))
    outs = [np.asarray(res.results[i]["out"]) for i in range(N_CORES)]
    attn = np.concatenate(outs, axis=0)  # [32, 2048]
    return attn[:, None, :].astype(np.float32)  # [32, 1, 2048]
